# revision 6
# baseline (speedup 1.0000x reference)
"""Trainium2 Bass kernel for MetaDynamics potential evaluation.

out[p] = sum_h hgt[h] * exp(-0.5 * sum_d (cen[h,d]-col[p,d])^2 / wdt[h,d]^2)
with H=16384 hills, P=4096 points, D=8 collective variables.

Algorithm: expand the quadratic form into a rank-17 inner product
  e'[h,p] = sum_d (cen*c)[h,d]*col[p,d] - 0.5*sum_d c[h,d]*col[p,d]^2 - 0.5*a[h]
  c = 1/wdt^2, a[h] = sum_d cen^2*c - 2*ln(hgt[h]);   out[p] = sum_h exp(e'[h,p])
so e' = W~ @ F^T is a K=17 matmul (W~=[cen*c, -c/2, -a/2], F=[col, col^2, 1]).

The matmul is PRE-SCALED into Schraudolph form: it directly computes
  y[h,p] = e'[h,p] * 2^23/ln2 + (127 - CSH) * 2^23
so that
  - the ACT engine recovers exact exp via its free affine:
      exp(SIG*y + BETA) = exp(e'), SIG = ln2/2^23, BETA = -(127-CSH)*ln2
  - the Vector engine gets a fast approximate exp: int32(max(y,0)) bit-cast
    to float32 IS ~exp(e') (Schraudolph), with CSH calibrated for zero-mean
    relative error so hill-sums retain ~0.3% accuracy (gate is 2e-2).

Precision: both factors split into bf16 hi+lo parts stacked to K=51
(lhsT rows [Fhi;Flo;Fhi] x rhs rows [Whi;Whi;Wlo]) reproducing the fp32
product to ~2^-17 while streaming the PE at full bf16 rate.

Sharding: hills split across 8 NeuronCores (2048 each); each core returns two
partial [4096] potentials (exact part + approx part); host sums all 16.

Per-core dataflow: for each of 32 point-tiles (128 points):
  4x matmul [K=51, M=128, N=512] -> one [128, 2048] PSUM tile (4 banks)
  ACT: EXP over hills [0:HA) -> SBUF trash + accum -> acca[:, i]   (exact)
  DVE: converts hills [HA:2048) to int32 Schraudolph bits (1x)
  GPS: pairwise-folds the HD bitcast floats to HD/2 (tensor_tensor add)
  DVE: tensor_tensor_reduce folds HD/2 -> HD/4 and accumulates -> accd[:, i]
EXP and the convert both only READ the PSUM tile so they run concurrently;
the fold/reduce chain trails one tile behind. PSUM ping-pongs 2 sets of 4
banks. The last tile is ACT-only so the approx chain doesn't extend the tail.
"""

import numpy as np
import ml_dtypes

import concourse.bacc as bacc
import concourse.mybir as mybir
import concourse.tile as tile
from concourse import bass_utils

H, P, D = 16384, 4096, 8
NCORES = 8
HL = H // NCORES          # hills per core
K = 51                    # 3 x 17 stacked hi/lo blocks
PT = 128                  # points per tile (PSUM partitions)
NPT = P // PT             # 32 p-tiles
HC = 512                  # hills per matmul (one PSUM bank of f32)
NHC = HL // HC            # 4 matmuls per p-tile
NMM = 512                 # free-dim per matmul instruction (1 PSUM bank max)

HA = 1064                 # hills per tile handled exactly by ACT
HD = HL - HA              # hills per tile handled by DVE fast-exp
HF = HD // 2              # after the GpSimd pairwise fold
HQ = HF // 2              # after the DVE ttr fold

LN2 = float(np.log(2.0))
CSH = 0.0575395
SIG = LN2 / (1 << 23)               # ACT recovery scale
BETA = -(127.0 - CSH) * LN2         # ACT recovery bias
SSCALE = (1 << 23) / LN2            # matmul pre-scale
SOFF = (127.0 - CSH) * (1 << 23)    # matmul constant-row offset

BF16 = mybir.dt.bfloat16
F32 = mybir.dt.float32
I32 = mybir.dt.int32

_NC_CACHE = None


def _build_nc():
    nc = bacc.Bacc(
        "TRN2",
        target_bir_lowering=False,
        debug=False,
        enable_asserts=False,
        num_devices=NCORES,
    )
    ft = nc.dram_tensor("ft", [K, P], BF16, kind="ExternalInput").ap()
    wt = nc.dram_tensor("wt", [K, HL], BF16, kind="ExternalInput").ap()
    # outa/outd[p_lane, n_tile]: row-major so the final DMA writes per-partition
    # runs. Host computes (outa+outd).T.ravel() for the [4096] point order.
    outa = nc.dram_tensor("outa", [PT, NPT], F32, kind="ExternalOutput").ap()
    outd = nc.dram_tensor("outd", [PT, NPT], F32, kind="ExternalOutput").ap()

    with tile.TileContext(nc) as tc:
        with (
            tc.tile_pool(name="const", bufs=1) as cpool,
            tc.tile_pool(name="cvt", bufs=2) as vpool,
            tc.tile_pool(name="fold", bufs=2) as fpool,
            tc.tile_pool(name="psum", bufs=2, space="PSUM") as ppool,
        ):
            ftt = cpool.tile([K, P], BF16)
            wtt = cpool.tile([K, HL], BF16)
            acca = cpool.tile([PT, NPT], F32)
            accd = cpool.tile([PT, NPT], F32)
            biast = cpool.tile([PT, 1], F32)
            atrash = cpool.tile([PT, HL], BF16)
            ttrash = cpool.tile([PT, HQ], F32)

            nc.gpsimd.memset(biast[:], BETA)
            # the last tile is ACT-only; its accd column stays zero
            nc.vector.memset(accd[:, NPT - 1 : NPT], 0.0)

            # Input DMAs on the two HWDGE engines whose sequencers are free
            # early (sync; scalar before its EXP stream starts), ordered so
            # ft[:,0:PT] + wt chunks 0-2 (which gate the first EXP) land first.
            nc.scalar.dma_start(ftt[:, 0:PT], ft[:, 0:PT])
            nc.sync.dma_start(wtt[:, 0:HC], wt[:, 0:HC])
            nc.scalar.dma_start(wtt[:, 2 * HC : 3 * HC], wt[:, 2 * HC : 3 * HC])
            nc.sync.dma_start(wtt[:, HC : 2 * HC], wt[:, HC : 2 * HC])
            nc.scalar.dma_start(wtt[:, 3 * HC : HL], wt[:, 3 * HC : HL])
            nc.sync.dma_start(ftt[:, PT:1408], ft[:, PT:1408])
            nc.scalar.dma_start(ftt[:, 1408:2752], ft[:, 1408:2752])
            nc.sync.dma_start(ftt[:, 2752:P], ft[:, 2752:P])

            folds = []
            for i in range(NPT):
                pt = ppool.tile([PT, HL], F32)  # 4 PSUM banks
                for j in range(HL // NMM):
                    nc.tensor.matmul(
                        pt[:, j * NMM : (j + 1) * NMM],
                        lhsT=ftt[:, i * PT : (i + 1) * PT],
                        rhs=wtt[:, j * NMM : (j + 1) * NMM],
                        start=True,
                        stop=True,
                    )
                ha = HL if i == NPT - 1 else HA
                # exact part on ACT (fused exp + sum over hills [0:ha)); the
                # elementwise output goes to an SBUF trash tile so the PSUM
                # tile is only READ -> DVE converts concurrently.
                nc.scalar.activation(
                    atrash[:, 0:ha],
                    pt[:, 0:ha],
                    mybir.ActivationFunctionType.Exp,
                    scale=SIG,
                    bias=biast[:],
                    accum_out=acca[:, i : i + 1],
                )
                if i < NPT - 1:
                    # approx part: DVE converts+clamps to int32 Schraudolph bits
                    cvt = vpool.tile([PT, HD], I32)
                    nc.vector.tensor_scalar(
                        cvt[:], pt[:, HA:HL], 0.0, None, mybir.AluOpType.max
                    )
                    # GPS pairwise-folds HD -> HF
                    fold = fpool.tile([PT, HF], F32)
                    nc.gpsimd.tensor_tensor(
                        fold[:],
                        cvt[:, 0:HF].bitcast(F32),
                        cvt[:, HF:HD].bitcast(F32),
                        mybir.AluOpType.add,
                    )
                    folds.append(fold)
                # DVE folds+accumulates the previous tile's GPS fold so that
                # chain is never on the PSUM critical path
                if 0 < i < NPT:
                    f = folds[i - 1]
                    nc.vector.tensor_tensor_reduce(
                        ttrash[:],
                        f[:, 0:HQ],
                        f[:, HQ:HF],
                        1.0,
                        0.0,
                        mybir.AluOpType.add,
                        mybir.AluOpType.add,
                        accum_out=accd[:, i - 1 : i],
                    )
                if i == NPT // 2 - 1:
                    nc.sync.dma_start(outa[:, : NPT // 2], acca[:, : NPT // 2])
                if i == NPT // 2 + 1:
                    nc.sync.dma_start(outd[:, : NPT // 2], accd[:, : NPT // 2])
            # final outd half: desc-gen on gpsimd hides under the last EXP
            nc.gpsimd.dma_start(outd[:, NPT // 2 :], accd[:, NPT // 2 :])
            nc.sync.dma_start(outa[:, NPT // 2 :], acca[:, NPT // 2 :])

    nc.compile()
    return nc


def _get_nc():
    global _NC_CACHE
    if _NC_CACHE is None:
        _NC_CACHE = _build_nc()
    return _NC_CACHE


def _split_bf16(x64):
    hi = x64.astype(ml_dtypes.bfloat16)
    lo = (x64 - hi.astype(np.float64)).astype(ml_dtypes.bfloat16)
    return hi, lo


def _prepare_inputs(col, cen, wdt, hgt):
    col64 = col.astype(np.float64)
    cen64 = cen.astype(np.float64)
    wdt64 = wdt.astype(np.float64)
    hgt64 = np.maximum(hgt.astype(np.float64), 1e-38)

    c = 1.0 / (wdt64 * wdt64)                                     # [H, D]
    a = np.sum(cen64 * cen64 * c, axis=1) - 2.0 * np.log(hgt64)   # [H]
    W = np.concatenate([cen64 * c, -0.5 * c, -0.5 * a[:, None]], axis=1)  # [H, 17]
    W *= SSCALE
    W[:, 16] += SOFF
    F = np.concatenate([col64, col64 * col64, np.ones((P, 1))], axis=1)   # [P, 17]

    Whi, Wlo = _split_bf16(W)
    Fhi, Flo = _split_bf16(F)

    ft = np.ascontiguousarray(np.concatenate([Fhi.T, Flo.T, Fhi.T], axis=0))  # [51, P]
    wt_full = np.concatenate([Whi.T, Whi.T, Wlo.T], axis=0)                   # [51, H]
    wts = [
        np.ascontiguousarray(wt_full[:, i * HL : (i + 1) * HL]) for i in range(NCORES)
    ]
    return ft, wts


def run_on_hw(col, cen, wdt, hgt, trace=False):
    """Run the SPMD kernel on 8 cores; returns (out[P] f32, BassKernelResults)."""
    ft, wts = _prepare_inputs(col, cen, wdt, hgt)
    nc = _get_nc()
    in_maps = [{"ft": ft, "wt": wts[i]} for i in range(NCORES)]
    res = bass_utils.run_bass_kernel_spmd(
        nc, in_maps, core_ids=list(range(NCORES)), trace=trace
    )
    total = np.zeros(P, dtype=np.float64)
    for r in res.results:
        total += r["outa"].T.reshape(P).astype(np.float64)
        total += r["outd"].T.reshape(P).astype(np.float64)
    return total.astype(np.float32), res


def kernel(col, cen, wdt, hgt):
    out, _ = run_on_hw(col, cen, wdt, hgt, trace=False)
    return out


# revision 7
# speedup vs baseline: 1.0353x; 1.0353x over previous
"""Trainium2 Bass kernel for MetaDynamics potential evaluation.

out[p] = sum_h hgt[h] * exp(-0.5 * sum_d (cen[h,d]-col[p,d])^2 / wdt[h,d]^2)
with H=16384 hills, P=4096 points, D=8 collective variables.

Algorithm: expand the quadratic form into a rank-17 inner product
  e'[h,p] = sum_d (cen*c)[h,d]*col[p,d] - 0.5*sum_d c[h,d]*col[p,d]^2 - 0.5*a[h]
  c = 1/wdt^2, a[h] = sum_d cen^2*c - 2*ln(hgt[h]);   out[p] = sum_h exp(e'[h,p])
so e' = W~ @ F^T is a K=17 matmul (W~=[cen*c, -c/2, -a/2], F=[col, col^2, 1]).

The matmul is PRE-SCALED into Schraudolph form: it directly computes
  y[h,p] = e'[h,p] * 2^23/ln2 + (127 - CSH) * 2^23
so that
  - the ACT engine recovers exact exp via its free affine:
      exp(SIG*y + BETA) = exp(e'), SIG = ln2/2^23, BETA = -(127-CSH)*ln2
  - the Vector engine gets a fast approximate exp: int32(max(y,0)) bit-cast
    to float32 IS ~exp(e') (Schraudolph), with CSH calibrated for zero-mean
    relative error so hill-sums retain ~0.3% accuracy (gate is 2e-2).

Precision: both factors split into bf16 hi+lo parts stacked to K=51
(lhsT rows [Fhi;Flo;Fhi] x rhs rows [Whi;Whi;Wlo]) reproducing the fp32
product to ~2^-17 while streaming the PE at full bf16 rate.

Sharding: hills split across 8 NeuronCores (2048 each); each core returns two
partial [4096] potentials (exact part + approx part); host sums all 16.

Per-core dataflow: for each of 32 point-tiles (128 points):
  4x matmul [K=51, M=128, N=512] -> one [128, 2048] PSUM tile (4 banks)
  ACT: EXP over hills [0:HA) in-place + accum -> acca[:, i]       (exact)
  DVE: converts hills [HA:2048) to int32 Schraudolph bits (1x)
  GPS: pairwise-folds the HD bitcast floats to HF (tensor_tensor add)
  DVE: accumulates the fold of tile i-2 -> accd[:, i-2]           (1x accum)
The accumulate trails TWO tiles behind the convert so the DVE never waits
on the GPS fold (its latency is ~1.3us after ts1); EXP and the convert
run concurrently on the same PSUM set. The last tile is ACT-only so the
approx chain doesn't extend the tail (host ignores outd's last column).
"""

import numpy as np
import ml_dtypes

import concourse.bacc as bacc
import concourse.mybir as mybir
import concourse.tile as tile
from concourse import bass_utils

H, P, D = 16384, 4096, 8
NCORES = 8
HL = H // NCORES          # hills per core
K = 51                    # 3 x 17 stacked hi/lo blocks
PT = 128                  # points per tile (PSUM partitions)
NPT = P // PT             # 32 p-tiles
HC = 512                  # hills per matmul (one PSUM bank of f32)
NHC = HL // HC            # 4 matmuls per p-tile

HA = 1140                 # hills per tile handled exactly by ACT
HD = HL - HA              # hills per tile handled by DVE fast-exp (even)
HF = HD // 2              # after the GpSimd pairwise fold
EXP_INPLACE = True        # EXP writes back into the PSUM region it reads

LN2 = float(np.log(2.0))
CSH = 0.0575395
SIG = LN2 / (1 << 23)               # ACT recovery scale
BETA = -(127.0 - CSH) * LN2         # ACT recovery bias
SSCALE = (1 << 23) / LN2            # matmul pre-scale
SOFF = (127.0 - CSH) * (1 << 23)    # matmul constant-row offset

BF16 = mybir.dt.bfloat16
F32 = mybir.dt.float32
I32 = mybir.dt.int32

_NC_CACHE = None


def _build_nc():
    nc = bacc.Bacc(
        "TRN2",
        target_bir_lowering=False,
        debug=False,
        enable_asserts=False,
        num_devices=NCORES,
    )
    ft = nc.dram_tensor("ft", [K, P], BF16, kind="ExternalInput").ap()
    wt = nc.dram_tensor("wt", [K, HL], BF16, kind="ExternalInput").ap()
    # outa/outd[p_lane, n_tile]: row-major so the final DMA writes per-partition
    # runs. Host computes (outa+outd).T.ravel() for the [4096] point order.
    outa = nc.dram_tensor("outa", [PT, NPT], F32, kind="ExternalOutput").ap()
    outd = nc.dram_tensor("outd", [PT, NPT], F32, kind="ExternalOutput").ap()

    with tile.TileContext(nc) as tc:
        with (
            tc.tile_pool(name="const", bufs=1) as cpool,
            tc.tile_pool(name="cvt", bufs=2) as vpool,
            tc.tile_pool(name="fold", bufs=3) as fpool,
            tc.tile_pool(name="psum", bufs=2, space="PSUM") as ppool,
        ):
            ftt = cpool.tile([K, P], BF16)
            wtt = cpool.tile([K, HL], BF16)
            acca = cpool.tile([PT, NPT], F32)
            accd = cpool.tile([PT, NPT], F32)
            biast = cpool.tile([PT, 1], F32)
            atrash = cpool.tile([PT, HL], BF16)
            ttrash = cpool.tile([PT, HF], F32)

            nc.gpsimd.memset(biast[:], BETA)

            # Input DMAs: ft[:,0:PT] small and first on scalar (gates all
            # matmuls); wt on sync (first chunk alone); rest split across both.
            nc.scalar.dma_start(ftt[:, 0:PT], ft[:, 0:PT])
            nc.sync.dma_start(wtt[:, 0:HC], wt[:, 0:HC])
            nc.scalar.dma_start(wtt[:, HC:HL], wt[:, HC:HL])
            nc.sync.dma_start(ftt[:, PT:1408], ft[:, PT:1408])
            nc.scalar.dma_start(ftt[:, 1408:2752], ft[:, 1408:2752])
            nc.sync.dma_start(ftt[:, 2752:P], ft[:, 2752:P])

            def accum_fold(idx, fold_tile):
                nc.vector.tensor_scalar(
                    ttrash[:],
                    fold_tile[:],
                    1.0,
                    0.0,
                    mybir.AluOpType.mult,
                    mybir.AluOpType.add,
                    accum_out=accd[:, idx : idx + 1],
                )

            folds = []
            for i in range(NPT):
                pt = ppool.tile([PT, HL], F32)  # 4 PSUM banks
                for j in range(NHC):
                    nc.tensor.matmul(
                        pt[:, j * HC : (j + 1) * HC],
                        lhsT=ftt[:, i * PT : (i + 1) * PT],
                        rhs=wtt[:, j * HC : (j + 1) * HC],
                        start=True,
                        stop=True,
                    )
                ha = HL if i == NPT - 1 else HA
                # exact part on ACT (fused exp + sum over hills [0:ha))
                nc.scalar.activation(
                    pt[:, 0:ha] if EXP_INPLACE else atrash[:, 0:ha],
                    pt[:, 0:ha],
                    mybir.ActivationFunctionType.Exp,
                    scale=SIG,
                    bias=biast[:],
                    accum_out=acca[:, i : i + 1],
                )
                if i < NPT - 1:
                    # approx part: DVE converts+clamps to int32 Schraudolph bits
                    cvt = vpool.tile([PT, HD], I32)
                    nc.vector.tensor_scalar(
                        cvt[:], pt[:, HA:HL], 0.0, None, mybir.AluOpType.max
                    )
                    # GPS pairwise-folds HD -> HF
                    fold = fpool.tile([PT, HF], F32)
                    nc.gpsimd.tensor_tensor(
                        fold[:],
                        cvt[:, 0:HF].bitcast(F32),
                        cvt[:, HF:HD].bitcast(F32),
                        mybir.AluOpType.add,
                    )
                    folds.append(fold)
                # DVE accumulates the fold of tile i-2 (never waits on GPS)
                if i >= 2:
                    accum_fold(i - 2, folds[i - 2])
                if i == NPT // 2 - 1:
                    nc.sync.dma_start(outa[:, : NPT // 2], acca[:, : NPT // 2])
                if i == NPT // 2 + 3:
                    nc.sync.dma_start(outd[:, : NPT // 2], accd[:, : NPT // 2])
            accum_fold(NPT - 3, folds[NPT - 3])
            accum_fold(NPT - 2, folds[NPT - 2])
            # desc-gen for the outd tail overlaps the last (ACT-only) tile
            nc.gpsimd.dma_start(
                outd[:, NPT // 2 : NPT - 1], accd[:, NPT // 2 : NPT - 1]
            )
            nc.sync.dma_start(outa[:, NPT // 2 :], acca[:, NPT // 2 :])

    nc.compile()
    return nc


def _get_nc():
    global _NC_CACHE
    if _NC_CACHE is None:
        _NC_CACHE = _build_nc()
    return _NC_CACHE


def _split_bf16(x64):
    hi = x64.astype(ml_dtypes.bfloat16)
    lo = (x64 - hi.astype(np.float64)).astype(ml_dtypes.bfloat16)
    return hi, lo


def _prepare_inputs(col, cen, wdt, hgt):
    col64 = col.astype(np.float64)
    cen64 = cen.astype(np.float64)
    wdt64 = wdt.astype(np.float64)
    hgt64 = np.maximum(hgt.astype(np.float64), 1e-38)

    c = 1.0 / (wdt64 * wdt64)                                     # [H, D]
    a = np.sum(cen64 * cen64 * c, axis=1) - 2.0 * np.log(hgt64)   # [H]
    W = np.concatenate([cen64 * c, -0.5 * c, -0.5 * a[:, None]], axis=1)  # [H, 17]
    W *= SSCALE
    W[:, 16] += SOFF
    F = np.concatenate([col64, col64 * col64, np.ones((P, 1))], axis=1)   # [P, 17]

    Whi, Wlo = _split_bf16(W)
    Fhi, Flo = _split_bf16(F)

    ft = np.ascontiguousarray(np.concatenate([Fhi.T, Flo.T, Fhi.T], axis=0))  # [51, P]
    wt_full = np.concatenate([Whi.T, Whi.T, Wlo.T], axis=0)                   # [51, H]
    wts = [
        np.ascontiguousarray(wt_full[:, i * HL : (i + 1) * HL]) for i in range(NCORES)
    ]
    return ft, wts


def run_on_hw(col, cen, wdt, hgt, trace=False):
    """Run the SPMD kernel on 8 cores; returns (out[P] f32, BassKernelResults)."""
    ft, wts = _prepare_inputs(col, cen, wdt, hgt)
    nc = _get_nc()
    in_maps = [{"ft": ft, "wt": wts[i]} for i in range(NCORES)]
    res = bass_utils.run_bass_kernel_spmd(
        nc, in_maps, core_ids=list(range(NCORES)), trace=trace
    )
    total = np.zeros(P, dtype=np.float64)
    for r in res.results:
        total += r["outa"].T.reshape(P).astype(np.float64)
        # the last p-tile is handled entirely by the exact path; its outd
        # column was never written on device
        od = r["outd"].astype(np.float64).copy()
        od[:, NPT - 1] = 0.0
        total += od.T.reshape(P)
    return total.astype(np.float32), res


def kernel(col, cen, wdt, hgt):
    out, _ = run_on_hw(col, cen, wdt, hgt, trace=False)
    return out
